# revision 1
# baseline (speedup 1.0000x reference)
"""Trainium2 Bass kernel for Expansion + CPSDropout.

Computes, for x[4,256,64,64] f32 and rand_vals[320,320] f32:
    xp   = zero-pad x spatially by 2            -> [b,c,68,68]
    out[b,c,5i+p,5j+q] = xp[b,c,i+p,j+q] * M[5i+p,5j+q]
    M    = (rand_vals > 0.25, forced True at [2::5,2::5]) / 0.75

Strategy (8 cores, data parallel over the 1024 (b,c) channels, 128/core):
  - host precomputes a *binary* bf16 mask (exact 0/1); the 1/0.75 scale is
    folded into the DVE scalar_tensor_tensor op.
  - per core: x shard is pre-expanded along W once into
    xq[n, h, 5j+q] = xpad[n, h, j+q]   ([128 part, 68*320] f32 in SBUF)
    via 5 strided ScalarE copies + tiny border memsets.
  - TensorE broadcasts mask rows across all 128 partitions into PSUM via
    ones[1,128]^T @ mask_chunk[1,N] matmuls (bf16 in, exact f32 0/1 out).
  - VectorE computes out[n,a,J] = (xq[n, i+a, J] * 4/3) * mask_psum[a,J]
    per 5-output-row tile: the H-expansion is folded into in0's access
    pattern ((320,5 rows),(1,320)) so each output element costs one DVE
    element at fp32 1x.
  - DMA stores stream the 52.4MB/core result; kernel is HBM-write bound
    (~146us roofline at 358 GB/s per core).
"""

import numpy as np
import ml_dtypes

import concourse.bass as bass
import concourse.bacc as bacc
import concourse.mybir as mybir
import concourse.tile as tile
from concourse.bass_utils import run_bass_kernel_spmd

P = 128            # partitions = channels per core
N_CORES = 8
H = W = 64
S = 5              # stride
S2 = S // 2        # pad = 2
HP = H + 2 * S2    # 68
OUT_HW = H * S     # 320
OUT_ELEMS = OUT_HW * OUT_HW  # 102400
XQ_F = HP * OUT_HW           # 21760 f32 per partition
RATE = 0.25
SCALE = float(np.float32(1.0) / np.float32(1.0 - RATE))

I_PER_G = 2                  # i-tiles per store group
GROUPS = H // I_PER_G        # 32
TILE_F = S * OUT_HW          # 1600 f32 per i-tile (5 output rows)

_CACHE = {}


def _build_nc():
    nc = bacc.Bacc("TRN2", target_bir_lowering=False)
    x_t = nc.dram_tensor("x", [P, H * W], mybir.dt.float32, kind="ExternalInput")
    m_t = nc.dram_tensor(
        "mask", [OUT_HW, OUT_HW], mybir.dt.bfloat16, kind="ExternalInput"
    )
    o_t = nc.dram_tensor("out", [P, OUT_ELEMS], mybir.dt.float32, kind="ExternalOutput")

    with tile.TileContext(nc) as tc:
        with (
            tc.tile_pool(name="const", bufs=1) as constp,
            tc.tile_pool(name="xbuf", bufs=1) as xbufp,
            tc.tile_pool(name="mstage", bufs=2) as mstp,
            tc.tile_pool(name="obuf", bufs=3) as obufp,
            tc.tile_pool(name="mpsum", bufs=2, space="PSUM") as psump,
        ):
            ones_bf = constp.tile([1, P], mybir.dt.bfloat16)
            nc.vector.memset(ones_bf[:], 1.0)

            xstage = xbufp.tile([P, H * W], mybir.dt.float32)
            nc.gpsimd.dma_start(out=xstage[:], in_=x_t[:])
            x3 = xstage[:].rearrange("p (h w) -> p h w", h=H, w=W)

            # xq[n, h, 5j+q] = xpad[n, h, j+q]; border cells zeroed explicitly
            xq = xbufp.tile([P, XQ_F], mybir.dt.float32)
            xq3 = xq[:].rearrange("p (h J) -> p h J", h=HP, J=OUT_HW)
            xq4 = xq[:].rearrange("p (h j q) -> p h j q", h=HP, j=H, q=S)
            # top/bottom 2 padded rows
            nc.gpsimd.memset(xq3[:, 0:S2, :], 0.0)
            nc.gpsimd.memset(xq3[:, HP - S2 : HP, :], 0.0)
            # left/right columns that read padded W positions:
            # invalid (j,q): (0,0),(1,0),(0,1) -> J in {0,5,1}; (63,3),(62,4),(63,4)
            # -> J in {318,314,319}
            mid = xq3[:, S2 : S2 + H, :]
            nc.gpsimd.memset(mid[:, :, 0:2], 0.0)
            nc.gpsimd.memset(mid[:, :, 5:6], 0.0)
            nc.gpsimd.memset(mid[:, :, 314:315], 0.0)
            nc.gpsimd.memset(mid[:, :, 318:320], 0.0)
            # 5 strided copies per half (split by h so tile 0 is ready early)
            for h0, h1 in ((0, H // 2), (H // 2, H)):
                for q in range(S):
                    j_lo = max(0, S2 - q)
                    j_hi = min(W, W + S2 - q)  # exclusive
                    src_lo = j_lo + q - S2
                    nc.scalar.copy(
                        out=xq4[:, S2 + h0 : S2 + h1, j_lo:j_hi, q],
                        in_=x3[:, h0:h1, src_lo : src_lo + (j_hi - j_lo)],
                    )

            # mask rows grouped per store group: [32, 10*320]
            m_g = m_t[:].rearrange("(g r) c -> g (r c)", r=I_PER_G * S)

            xq_ap = xq[:]
            xq_pdim = list(xq_ap.ap[0])
            for g in range(GROUPS):
                mst = mstp.tile([1, I_PER_G * TILE_F], mybir.dt.bfloat16)
                nc.gpsimd.dma_start(out=mst[:], in_=m_g[g : g + 1, :])
                obuf = obufp.tile([P, I_PER_G * TILE_F], mybir.dt.float32)
                for u in range(I_PER_G):
                    i = g * I_PER_G + u
                    ps = psump.tile([P, TILE_F], mybir.dt.float32)
                    # broadcast 5 mask rows (1600 els) across 128 partitions
                    for j0 in range(0, TILE_F, 512):
                        j1 = min(TILE_F, j0 + 512)
                        nc.tensor.matmul(
                            ps[:, j0:j1],
                            ones_bf[:],
                            mst[0:1, u * TILE_F + j0 : u * TILE_F + j1],
                            start=True,
                            stop=True,
                        )
                    # fused H-expand+scale+mask:
                    #   out[n,a,J] = (xq[n, i+a, J] * 4/3) * m[5i+a, J]
                    in0 = bass.AP(
                        tensor=xq_ap.tensor,
                        offset=xq_ap.offset + i * OUT_HW,
                        ap=[xq_pdim, [OUT_HW, S], [1, OUT_HW]],
                    )
                    out_ap = obuf[:, u * TILE_F : (u + 1) * TILE_F].rearrange(
                        "p (a J) -> p a J", a=S
                    )
                    in1 = ps[:].rearrange("p (a J) -> p a J", a=S)
                    nc.vector.scalar_tensor_tensor(
                        out=out_ap,
                        in0=in0,
                        scalar=SCALE,
                        in1=in1,
                        op0=mybir.AluOpType.mult,
                        op1=mybir.AluOpType.mult,
                    )
                nc.sync.dma_start(
                    out=o_t[:, g * I_PER_G * TILE_F : (g + 1) * I_PER_G * TILE_F],
                    in_=obuf[:],
                )
    nc.compile()
    return nc


def _get_nc():
    if "nc" not in _CACHE:
        _CACHE["nc"] = _build_nc()
    return _CACHE["nc"]


def kernel(x: np.ndarray, rand_vals: np.ndarray, **run_kwargs) -> np.ndarray:
    b, c, h, w = x.shape
    assert (b, c, h, w) == (4, 256, 64, 64)
    n_total = b * c

    # binary keep-mask with forced keeps at patch centers, exact in bf16
    keep = np.asarray(rand_vals) > RATE
    keep[S2::S, S2::S] = True
    m01 = keep.astype(np.float32).astype(ml_dtypes.bfloat16)

    x_flat = np.ascontiguousarray(
        np.asarray(x).reshape(n_total, h * w).astype(np.float32, copy=False)
    )
    per_core = n_total // N_CORES
    in_maps = [
        {
            "x": x_flat[k * per_core : (k + 1) * per_core],
            "mask": m01,
        }
        for k in range(N_CORES)
    ]

    nc = _get_nc()
    res = run_bass_kernel_spmd(nc, in_maps, core_ids=list(range(N_CORES)), **run_kwargs)
    out = np.concatenate([r["out"] for r in res.results], axis=0)
    _CACHE["last_results"] = res
    return out.reshape(b, c, OUT_HW, OUT_HW)



# revision 4
# speedup vs baseline: 1.4356x; 1.4356x over previous
"""Trainium2 Bass kernel for Expansion + CPSDropout (v3, bf16 output).

Computes, for x[4,256,64,64] f32 and rand_vals[320,320] f32:
    out[b,c,5i+p,5j+q] = xpad[b,c,i+p,j+q] * M[5i+p,5j+q]
    M = (rand_vals > 0.25, forced True at [2::5,2::5]) * (4/3)

Strategy (8 cores, data parallel over the 1024 (b,c) channels, 128/core):
  - host: x*(4/3) -> bf16; binary mask {0,1} bf16 with zeros at the border
    positions whose source reads fall in the zero padding.
  - device: x is staged into a guard-padded buffer xg[128, 4360] so the TT
    input AP  out[a,j,q] = xg[64*(i+a) + j + q]  covers every output
    element of an i-tile (5 output rows) in one op; padding reads hit
    memset guards (H) or are killed by mask zeros (W wrap).
  - the mask must be replicated across partitions: the PE broadcasts each
    i-tile's 1600 mask values into PSUM via ones[1,128]^T @ mask[1,N]
    matmuls.  For 'A' groups ScalarE evacuates PSUM->SBUF bf16 and the
    DVE multiplies at its faster SBUF mode; for 'C' groups the DVE reads
    the mask directly from PSUM (saves ScalarE at a slower DVE rate).
  - output bf16 [128, 102400]/core, host upcasts to f32 (rel err ~2e-3,
    well under the 2e-2 gate).
"""

import numpy as np
import ml_dtypes

import concourse.bass as bass
import concourse.bacc as bacc
import concourse.mybir as mybir
import concourse.tile as tile
from concourse.bass_utils import run_bass_kernel_spmd

P = 128            # partitions = channels per core
N_CORES = 8
H = W = 64
S = 5
S2 = S // 2        # pad = 2
OUT_HW = H * S     # 320
OUT_ELEMS = OUT_HW * OUT_HW  # 102400
RATE = 0.25

XG_F = 68 * W + 8  # 4360: 2 guard rows + x + 2 guard rows + slack
XG_X0 = 2 * W + 2  # 130: offset of x[row 0, col 0] inside xg

I_PER_G = 4                  # i-tiles per store group
GROUPS = H // I_PER_G        # 16
TILE_F = S * OUT_HW          # 1600 per i-tile
G_F = I_PER_G * TILE_F       # 6400 per group

# 'A': PE->PSUM->ScalarE evac->SBUF, DVE TT from SBUF (fast DVE mode)
# 'C': PE->PSUM, DVE TT straight from PSUM (saves ScalarE, slower DVE)
PATTERN = "AACAACAACAACAAAA"

_CACHE = {}


def _build_nc():
    assert len(PATTERN) == GROUPS
    nc = bacc.Bacc("TRN2", target_bir_lowering=False)
    x_t = nc.dram_tensor("x", [P, H * W], mybir.dt.bfloat16, kind="ExternalInput")
    m_t = nc.dram_tensor("mask", [GROUPS, G_F], mybir.dt.bfloat16, kind="ExternalInput")
    o_t = nc.dram_tensor(
        "out", [P, OUT_ELEMS], mybir.dt.bfloat16, kind="ExternalOutput"
    )

    with tile.TileContext(nc) as tc:
        with (
            tc.tile_pool(name="const", bufs=1) as constp,
            tc.tile_pool(name="xbuf", bufs=1) as xbufp,
            tc.tile_pool(name="mstage", bufs=3) as mstp,
            tc.tile_pool(name="msb", bufs=3) as msbp,
            tc.tile_pool(name="obuf", bufs=4) as obufp,
            tc.tile_pool(name="mpsum", bufs=2, space="PSUM") as psump,
        ):
            ones_bf = constp.tile([1, P], mybir.dt.bfloat16)
            nc.vector.memset(ones_bf[:], 1.0)

            # guard-padded x: zero everything, then land x at offset 130
            xg = xbufp.tile([P, XG_F], mybir.dt.bfloat16)
            nc.scalar.memzero(xg[:])
            nc.gpsimd.dma_start(out=xg[:, XG_X0 : XG_X0 + H * W], in_=x_t[:])

            xg_ap = xg[:]
            xg_pdim = list(xg_ap.ap[0])
            TT_DIMS = [[OUT_HW, S], [S, H], [1, S]]  # (a, j, q) on 1600 elems

            def tt(i, out_tile, out_off, in1_tensor, in1_offset, in1_pdim):
                """out[a,j,q] = xg[64*(i+a) + j + q] * in1[a*320+5j+q]."""
                in0 = bass.AP(
                    tensor=xg_ap.tensor,
                    offset=xg_ap.offset + i * W,
                    ap=[xg_pdim, [W, S], [1, H], [1, S]],
                )
                o1 = bass.AP(
                    tensor=out_tile.tensor,
                    offset=out_tile.offset + out_off,
                    ap=[list(out_tile.ap[0])] + TT_DIMS,
                )
                i1 = bass.AP(
                    tensor=in1_tensor,
                    offset=in1_offset,
                    ap=[in1_pdim] + TT_DIMS,
                )
                nc.vector.tensor_tensor(
                    out=o1, in0=in0, in1=i1, op=mybir.AluOpType.mult
                )

            for g in range(GROUPS):
                path = PATTERN[g]
                obuf = obufp.tile([P, G_F], mybir.dt.bfloat16)
                mst = mstp.tile([1, G_F], mybir.dt.bfloat16)
                nc.gpsimd.dma_start(out=mst[:], in_=m_t[g : g + 1, :])
                if path == "A":
                    msb = msbp.tile([P, G_F], mybir.dt.bfloat16)
                for u in range(I_PER_G):
                    i = g * I_PER_G + u
                    ps = psump.tile([P, TILE_F], mybir.dt.float32)
                    for j0, j1 in ((0, 512), (512, 1024), (1024, 1536), (1536, 1600)):
                        nc.tensor.matmul(
                            ps[:, j0:j1],
                            ones_bf[:],
                            mst[0:1, u * TILE_F + j0 : u * TILE_F + j1],
                            start=True,
                            stop=True,
                        )
                    if path == "A":
                        nc.scalar.copy(
                            out=msb[:, u * TILE_F : (u + 1) * TILE_F], in_=ps[:]
                        )
                        msb_ap = msb[:]
                        tt(i, obuf[:], u * TILE_F,
                           msb_ap.tensor, msb_ap.offset + u * TILE_F,
                           list(msb_ap.ap[0]))
                    else:
                        ps_ap = ps[:]
                        tt(i, obuf[:], u * TILE_F,
                           ps_ap.tensor, ps_ap.offset, list(ps_ap.ap[0]))
                nc.sync.dma_start(out=o_t[:, g * G_F : (g + 1) * G_F], in_=obuf[:])
    nc.compile()
    return nc


def _get_nc():
    if "nc" not in _CACHE:
        _CACHE["nc"] = _build_nc()
    return _CACHE["nc"]


_BORDER = [0, 1, 5, OUT_HW - 6, OUT_HW - 2, OUT_HW - 1]  # source reads padding


def kernel(x: np.ndarray, rand_vals: np.ndarray, **run_kwargs) -> np.ndarray:
    b, c, h, w = x.shape
    assert (b, c, h, w) == (4, 256, 64, 64)
    n_total = b * c

    # binary keep-mask, forced keeps at centers, zeros where source = padding
    keep = np.asarray(rand_vals) > RATE
    keep[S2::S, S2::S] = True
    keep[_BORDER, :] = False
    keep[:, _BORDER] = False
    m01 = keep.astype(ml_dtypes.bfloat16).reshape(GROUPS, G_F)

    # 1/(1-rate) folded into x here (bf16 once)
    scale = np.float32(1.0) / np.float32(1.0 - RATE)
    x_bf = (np.asarray(x).reshape(n_total, h * w) * scale).astype(ml_dtypes.bfloat16)
    per_core = n_total // N_CORES
    in_maps = [
        {
            "x": np.ascontiguousarray(x_bf[k * per_core : (k + 1) * per_core]),
            "mask": m01,
        }
        for k in range(N_CORES)
    ]

    nc = _get_nc()
    res = run_bass_kernel_spmd(nc, in_maps, core_ids=list(range(N_CORES)), **run_kwargs)
    out = np.concatenate([r["out"] for r in res.results], axis=0)
    _CACHE["last_results"] = res
    return out.astype(np.float32).reshape(b, c, OUT_HW, OUT_HW)
